# revision 18
# baseline (speedup 1.0000x reference)
"""Distributed Trainium2 kernel for nn_BrocaNetwork (decoder transformer + 180k-vocab head).

Strategy (8 NeuronCores, two SPMD launches):
  1. Body launch (2 cores, one per batch): the 6-layer decoder runs fully
     on-device in bf16 (fp32 accumulation / layernorm stats). Activations are
     feature-major [D, tokens]; attention uses exp-then-causal-mask-multiply
     with per-head ones-column sum rows; cross-attention to the 1-token memory
     collapses exactly (softmax over one key == 1) to a precomputed per-batch
     vector added on device.
  2. Vocab projection launch (8 cores): w_out is sharded column-wise over the
     vocab (22500 rows/core, padded to 22528); each core computes
     x @ w_out_shard.T for all 1024 tokens in bf16 with fp32 PSUM accumulation.
Host work is limited to input re-layout: embedding row gather, weight
transposes/casts, and output concat.
"""
import sys
sys.path.insert(0, '/opt/trn_rl_repo')
import numpy as np
import ml_dtypes

import concourse.bass as bass
import concourse.mybir as mybir
import concourse.tile as tile
from concourse import bacc
from concourse.bass_utils import run_bass_kernel_spmd

F32 = mybir.dt.float32
BF16 = mybir.dt.bfloat16
Alu = mybir.AluOpType
Act = mybir.ActivationFunctionType
BF = ml_dtypes.bfloat16

B, S, D, H, L, V = 2, 512, 512, 8, 6, 180000
HD = D // H
FF = 4 * D
EPS = 1e-5
DT = D // 128
ST = S // 128
FT = FF // 128
NCORES = 8
VS = 22528              # padded vocab shard (44 * 512); 8 * 22528 >= V
TOK = B * S
NT = VS // 512
MT = TOK // 128


def build_body(n_cores):
    nc = bacc.Bacc("TRN2", target_bir_lowering=False, debug=False, num_devices=n_cores)
    x0T = nc.dram_tensor("x0T", [D, S], F32, kind="ExternalInput")
    wqT = nc.dram_tensor("wqT", [L, D, D], BF16, kind="ExternalInput")
    wkT = nc.dram_tensor("wkT", [L, D, D], BF16, kind="ExternalInput")
    wvT = nc.dram_tensor("wvT", [L, D, D], BF16, kind="ExternalInput")
    woT = nc.dram_tensor("woT", [L, D, D], BF16, kind="ExternalInput")
    w1T = nc.dram_tensor("w1T", [L, D, FF], BF16, kind="ExternalInput")
    w2T = nc.dram_tensor("w2T", [L, FF, D], BF16, kind="ExternalInput")
    bq = nc.dram_tensor("bq", [L, D], F32, kind="ExternalInput")
    bk = nc.dram_tensor("bk", [L, D], F32, kind="ExternalInput")
    abias = nc.dram_tensor("abias", [L, D], F32, kind="ExternalInput")
    cao = nc.dram_tensor("cao", [L, D], F32, kind="ExternalInput")
    b1 = nc.dram_tensor("b1", [L, FF], F32, kind="ExternalInput")
    b2 = nc.dram_tensor("b2", [L, D], F32, kind="ExternalInput")
    g1 = nc.dram_tensor("g1", [L, D], F32, kind="ExternalInput")
    bl1 = nc.dram_tensor("bl1", [L, D], F32, kind="ExternalInput")
    g2 = nc.dram_tensor("g2", [L, D], F32, kind="ExternalInput")
    bl2 = nc.dram_tensor("bl2", [L, D], F32, kind="ExternalInput")
    g3 = nc.dram_tensor("g3", [L, D], F32, kind="ExternalInput")
    bl3 = nc.dram_tensor("bl3", [L, D], F32, kind="ExternalInput")
    maskd = nc.dram_tensor("maskd", [128, 128], BF16, kind="ExternalInput")
    xfT = nc.dram_tensor("xfT", [D, S], BF16, kind="ExternalOutput")

    def vec_ap(t):
        return t.ap().rearrange("l (t p) -> p l t", p=128)

    with nc.allow_low_precision(reason="bf16 compute kernel"), tile.TileContext(nc) as tc:
        with tc.tile_pool(name="persist", bufs=1) as pp, \
             tc.tile_pool(name="wqkvo", bufs=2) as wpool, \
             tc.tile_pool(name="w1p", bufs=2) as w1pool, \
             tc.tile_pool(name="w2p", bufs=2) as w2pool, \
             tc.tile_pool(name="lntmp", bufs=2) as lnp, \
             tc.tile_pool(name="exps", bufs=8) as xp, \
             tc.tile_pool(name="psA", bufs=5, space="PSUM") as psA, \
             tc.tile_pool(name="psV", bufs=3, space="PSUM") as psV:

            x_res = pp.tile([128, DT, S], F32)
            x_bf = pp.tile([128, DT, S], BF16)
            q_bf = pp.tile([128, DT, S], BF16)
            k_bf = pp.tile([128, DT, S], BF16)
            v_sb = pp.tile([128, ST, H, HD + 1], BF16)
            o_bf = pp.tile([128, DT, S], BF16)
            hh_bf = pp.tile([128, FT, S], BF16)
            ones_bf = pp.tile([128, 128], BF16)
            mask_sb = pp.tile([128, 128], BF16)
            sums_sb = pp.tile([1, H, S], BF16)
            bq_sb = pp.tile([128, L, DT], F32)
            bk_sb = pp.tile([128, L, DT], F32)
            ab_sb = pp.tile([128, L, DT], F32)
            cao_sb = pp.tile([128, L, DT], F32)
            b1_sb = pp.tile([128, L, FT], F32)
            b2_sb = pp.tile([128, L, DT], F32)
            g1_sb = pp.tile([128, L, DT], F32)
            bl1_sb = pp.tile([128, L, DT], F32)
            g2_sb = pp.tile([128, L, DT], F32)
            bl2_sb = pp.tile([128, L, DT], F32)
            g3_sb = pp.tile([128, L, DT], F32)
            bl3_sb = pp.tile([128, L, DT], F32)

            eps_sb = pp.tile([128, 1], F32)
            nc.vector.memset(eps_sb[:], float(D * EPS))
            nc.vector.memset(ones_bf[:], 1.0)
            nc.vector.memset(v_sb[:, :, :, HD:HD + 1], 1.0)
            nc.sync.dma_start(x_res[:], x0T.ap().rearrange("(t p) n -> p t n", p=128))
            for k in range(DT):
                nc.vector.tensor_copy(x_bf[:, k, :], x_res[:, k, :])
            nc.gpsimd.dma_start(mask_sb[:], maskd.ap())
            nc.gpsimd.dma_start(bq_sb[:], vec_ap(bq))
            nc.gpsimd.dma_start(bk_sb[:], vec_ap(bk))
            nc.gpsimd.dma_start(ab_sb[:], vec_ap(abias))
            nc.gpsimd.dma_start(cao_sb[:], vec_ap(cao))
            nc.gpsimd.dma_start(b1_sb[:], b1.ap().rearrange("l (t p) -> p l t", p=128))
            nc.gpsimd.dma_start(b2_sb[:], vec_ap(b2))
            nc.gpsimd.dma_start(g1_sb[:], vec_ap(g1))
            nc.gpsimd.dma_start(bl1_sb[:], vec_ap(bl1))
            nc.gpsimd.dma_start(g2_sb[:], vec_ap(g2))
            nc.gpsimd.dma_start(bl2_sb[:], vec_ap(bl2))
            nc.gpsimd.dma_start(g3_sb[:], vec_ap(g3))
            nc.gpsimd.dma_start(bl3_sb[:], vec_ap(bl3))

            def layer_norm(P, g_ap, b_ap):
                Pbf = lnp.tile([128, DT, S], BF16, tag="pbf")
                Psq = lnp.tile([128, DT, S], BF16, tag="psq")
                for k in range(DT):
                    nc.vector.tensor_copy(Pbf[:, k, :], P[:, k, :])
                    nc.scalar.activation(Psq[:, k, :], P[:, k, :], Act.Square)
                A_ps = psA.tile([128, S], F32, tag="big")
                Q_ps = psA.tile([128, S], F32, tag="big")
                for k in range(DT):
                    nc.tensor.matmul(A_ps[:], ones_bf[:], Pbf[:, k, :], start=(k == 0), stop=(k == DT - 1))
                for k in range(DT):
                    nc.tensor.matmul(Q_ps[:], ones_bf[:], Psq[:, k, :], start=(k == 0), stop=(k == DT - 1))
                t_sb = lnp.tile([128, S], F32, tag="t")
                nc.scalar.activation(t_sb[:], A_ps[:], Act.Square)
                nc.vector.scalar_tensor_tensor(t_sb[:], t_sb[:], -1.0 / D, Q_ps[:], op0=Alu.mult, op1=Alu.add)
                nc.scalar.activation(t_sb[:], t_sb[:], Act.Abs_reciprocal_sqrt, bias=eps_sb[:])
                f_ps = psA.tile([128, S], F32, tag="big")
                for f in range(22):
                    nc.tensor.matmul(f_ps[:], ones_bf[:], Pbf[:, DT - 1, :],
                                     start=(f == 0), stop=(f == 21))
                for k in range(DT):
                    nc.vector.scalar_tensor_tensor(P[:, k, :], A_ps[:], -1.0 / D, P[:, k, :], op0=Alu.mult, op1=Alu.add)
                    nc.vector.tensor_mul(P[:, k, :], P[:, k, :], t_sb[:])
                    nc.scalar.activation(x_bf[:, k, :], P[:, k, :], Act.Identity, bias=b_ap(k), scale=g_ap(k))
                    nc.vector.tensor_scalar(x_res[:, k, :], P[:, k, :], g_ap(k), b_ap(k), op0=Alu.mult, op1=Alu.add)

            for l in range(L):
                wq_sb = wpool.tile([128, DT, D], BF16, tag="wq")
                wk_sb = wpool.tile([128, DT, D], BF16, tag="wk")
                wv_sb = wpool.tile([128, DT, D], BF16, tag="wv")
                wo_sb = wpool.tile([128, DT, D], BF16, tag="wo")
                w1_sb = w1pool.tile([128, DT, FF], BF16, tag="w1")
                w2_sb = w2pool.tile([128, FT, D], BF16, tag="w2")
                nc.sync.dma_start(wq_sb[:], wqT.ap()[l].rearrange("(t p) n -> p t n", p=128))
                nc.sync.dma_start(wk_sb[:], wkT.ap()[l].rearrange("(t p) n -> p t n", p=128))
                nc.sync.dma_start(wv_sb[:], wvT.ap()[l].rearrange("(t p) n -> p t n", p=128))
                nc.sync.dma_start(wo_sb[:], woT.ap()[l].rearrange("(t p) n -> p t n", p=128))
                nc.sync.dma_start(w1_sb[:], w1T.ap()[l].rearrange("(t p) n -> p t n", p=128))
                nc.sync.dma_start(w2_sb[:], w2T.ap()[l].rearrange("(t p) n -> p t n", p=128))

                for dst, w_sb, bias_sb in ((q_bf, wq_sb, bq_sb), (k_bf, wk_sb, bk_sb)):
                    for m in range(DT):
                        ps = psA.tile([128, S], F32, tag="big")
                        for k in range(DT):
                            nc.tensor.matmul(ps[:], w_sb[:, k, m * 128:(m + 1) * 128], x_bf[:, k, :],
                                             start=(k == 0), stop=(k == DT - 1))
                        nc.scalar.activation(dst[:, m, :], ps[:], Act.Identity, bias=bias_sb[:, l, m:m + 1])
                for t in range(ST):
                    ps = psA.tile([128, S], F32, tag="big")
                    for k in range(DT):
                        nc.tensor.matmul(ps[:], x_bf[:, k, t * 128:(t + 1) * 128], wv_sb[:, k, :],
                                         start=(k == 0), stop=(k == DT - 1))
                    nc.scalar.copy(v_sb[:, t, :, 0:HD], ps[:].rearrange("p (h d) -> p h d", h=H))
                av_tiles = []

                def attn_norm(h):
                    dt_i = h // 2
                    pb = 64 * (h % 2)
                    av_ps = av_tiles[h]
                    sb_ps = psV.tile([128, S], F32, tag="av")
                    nc.tensor.matmul(sb_ps[:], ones_bf[0:1, :], sums_sb[:, h, :], start=True, stop=True)
                    recip_sb = lnp.tile([128, S], F32, tag="recip")
                    nc.vector.reciprocal_approx_fast(out=recip_sb[0:64, :], in_=sb_ps[0:64, :])
                    nc.vector.tensor_mul(o_bf[pb:pb + HD, dt_i, :], av_ps[0:HD, :], recip_sb[0:HD, :])

                for h in range(H):
                    if h >= 2:
                        attn_norm(h - 2)
                    dt_i = h // 2
                    pb = 64 * (h % 2)
                    av_ps = psV.tile([HD + 1, S], F32, tag="av")
                    av_tiles.append(av_ps)
                    for i in range(ST):
                        w = S - 128 * i
                        sc = psA.tile([128, S], F32, tag="big")
                        nc.tensor.matmul(sc[:, 0:w],
                                         k_bf[pb:pb + HD, dt_i, i * 128:(i + 1) * 128],
                                         q_bf[pb:pb + HD, dt_i, i * 128:S],
                                         start=True, stop=True)
                        ex = xp.tile([128, S], BF16, tag="ex")
                        nc.scalar.activation(ex[:, 0:w], sc[:, 0:w], Act.Exp, scale=1.0 / np.sqrt(HD))
                        nc.vector.tensor_mul(ex[:, 0:128], ex[:, 0:128], mask_sb[:])
                        nc.tensor.matmul(av_ps[:, i * 128:S], v_sb[:, i, h, :], ex[:, 0:w],
                                         start=(i == 0), stop=(i == ST - 1), skip_group_check=True)
                    nc.vector.tensor_copy(sums_sb[:, h, :], av_ps[HD:HD + 1, :])
                for h in (H - 2, H - 1):
                    attn_norm(h)
                P = lnp.tile([128, DT, S], F32, tag="P")
                for m in range(DT):
                    ps = psA.tile([128, S], F32, tag="big")
                    for k in range(DT):
                        nc.tensor.matmul(ps[:], wo_sb[:, k, m * 128:(m + 1) * 128], o_bf[:, k, :],
                                         start=(k == 0), stop=(k == DT - 1))
                    nc.scalar.activation(P[:, m, :], ps[:], Act.Identity, bias=ab_sb[:, l, m:m + 1])
                for k in range(DT):
                    nc.vector.tensor_add(P[:, k, :], P[:, k, :], x_res[:, k, :])
                layer_norm(P, lambda k: g1_sb[:, l, k:k + 1], lambda k: bl1_sb[:, l, k:k + 1])

                P2 = lnp.tile([128, DT, S], F32, tag="P")
                for k in range(DT):
                    nc.vector.tensor_scalar_add(P2[:, k, :], x_res[:, k, :], cao_sb[:, l, k:k + 1])
                layer_norm(P2, lambda k: g2_sb[:, l, k:k + 1], lambda k: bl2_sb[:, l, k:k + 1])

                for m in range(FT):
                    ps = psA.tile([128, S], F32, tag="big")
                    for k in range(DT):
                        nc.tensor.matmul(ps[:], w1_sb[:, k, m * 128:(m + 1) * 128], x_bf[:, k, :],
                                         start=(k == 0), stop=(k == DT - 1))
                    nc.scalar.activation(hh_bf[:, m, :], ps[:], Act.Relu, bias=b1_sb[:, l, m:m + 1])
                P3 = lnp.tile([128, DT, S], F32, tag="P")
                for m in range(DT):
                    ps = psA.tile([128, S], F32, tag="big")
                    for k in range(FT):
                        nc.tensor.matmul(ps[:], w2_sb[:, k, m * 128:(m + 1) * 128], hh_bf[:, k, :],
                                         start=(k == 0), stop=(k == FT - 1))
                    nc.scalar.activation(P3[:, m, :], ps[:], Act.Identity, bias=b2_sb[:, l, m:m + 1])
                for k in range(DT):
                    nc.vector.tensor_add(P3[:, k, :], P3[:, k, :], x_res[:, k, :])
                layer_norm(P3, lambda k: g3_sb[:, l, k:k + 1], lambda k: bl3_sb[:, l, k:k + 1])

            nc.sync.dma_start(xfT.ap().rearrange("(t p) n -> p t n", p=128), x_bf[:])
    nc.compile()
    return nc


def build_proj(n_cores):
    nc = bacc.Bacc("TRN2", target_bir_lowering=False, debug=False, num_devices=n_cores)
    xT = nc.dram_tensor("xT", [D, TOK], BF16, kind="ExternalInput")
    wT = nc.dram_tensor("wT", [D, VS], BF16, kind="ExternalInput")
    out = nc.dram_tensor("out", [TOK, VS], BF16, kind="ExternalOutput")
    with tile.TileContext(nc) as tc:
        with tc.tile_pool(name="xp", bufs=1) as xpool, \
             tc.tile_pool(name="wp", bufs=8) as wpool, \
             tc.tile_pool(name="op", bufs=6) as opool, \
             tc.tile_pool(name="ps", bufs=8, space="PSUM") as psp:
            x_sb = xpool.tile([128, DT, TOK], BF16)
            nc.sync.dma_start(x_sb[:], xT.ap().rearrange("(t p) n -> p t n", p=128))
            warm = psp.tile([128, 512], F32, tag="ps")
            for f in range(20):
                nc.tensor.matmul(warm[:], x_sb[:, 0, 0:128], x_sb[:, 0, 0:512],
                                 start=(f == 0), stop=(f == 19))
            for nt in range(NT):
                w_sb = wpool.tile([128, DT, 512], BF16)
                nc.sync.dma_start(w_sb[:], wT.ap()[:, nt * 512:(nt + 1) * 512].rearrange("(t p) n -> p t n", p=128))
                for mt in range(MT):
                    ps = psp.tile([128, 512], F32, tag="ps")
                    for kt in range(DT):
                        nc.tensor.matmul(ps[:], x_sb[:, kt, mt * 128:(mt + 1) * 128], w_sb[:, kt, :],
                                         start=(kt == 0), stop=(kt == DT - 1))
                    ob = opool.tile([128, 512], BF16)
                    if mt % 2 == 0:
                        nc.scalar.copy(ob[:], ps[:])
                    else:
                        nc.vector.tensor_copy(ob[:], ps[:])
                    nc.sync.dma_start(out.ap()[mt * 128:(mt + 1) * 128, nt * 512:(nt + 1) * 512], ob[:])
    nc.compile()
    return nc


_CACHE = {}


def _get_kernels():
    if "body" not in _CACHE:
        _CACHE["body"] = build_body(B)
        _CACHE["proj"] = build_proj(NCORES)
    return _CACHE["body"], _CACHE["proj"]


def _body_in_map(x0, sa_wq, sa_wk, sa_wv, sa_bq, sa_bk, sa_bv, sa_wo, sa_bo,
                 ca_wv, ca_bv, ca_wo, ca_bo, meaning_b,
                 ln1_g, ln1_b, ln2_g, ln2_b, ln3_g, ln3_b,
                 ff_w1, ff_b1, ff_w2, ff_b2, mask):
    tp = lambda w: np.ascontiguousarray(w.transpose(0, 2, 1)).astype(BF)
    cao = np.stack([
        (meaning_b @ ca_wv[l].T + ca_bv[l]) @ ca_wo[l].T + ca_bo[l] for l in range(L)])
    abias = np.stack([sa_bo[l] + sa_wo[l] @ sa_bv[l] for l in range(L)])
    sq = np.float32(np.sqrt(D))
    f32 = lambda a: np.ascontiguousarray(a, dtype=np.float32)
    return {
        "x0T": np.ascontiguousarray(x0.T).astype(np.float32),
        "wqT": tp(sa_wq), "wkT": tp(sa_wk), "wvT": tp(sa_wv), "woT": tp(sa_wo),
        "w1T": tp(ff_w1), "w2T": tp(ff_w2),
        "bq": f32(sa_bq), "bk": f32(sa_bk), "abias": f32(abias), "cao": f32(cao),
        "b1": f32(ff_b1), "b2": f32(ff_b2),
        "g1": f32(ln1_g * sq), "bl1": f32(ln1_b),
        "g2": f32(ln2_g * sq), "bl2": f32(ln2_b),
        "g3": f32(ln3_g * sq), "bl3": f32(ln3_b),
        "maskd": mask,
    }


def kernel(meaning, target_ids, emb_table, pos_table,
           sa_wq, sa_wk, sa_wv, sa_bq, sa_bk, sa_bv, sa_wo, sa_bo,
           ca_wq, ca_wk, ca_wv, ca_bq, ca_bk, ca_bv, ca_wo, ca_bo,
           ln1_g, ln1_b, ln2_g, ln2_b, ln3_g, ln3_b,
           ff_w1, ff_b1, ff_w2, ff_b2, w_out, b_out):
    meaning = np.asarray(meaning, dtype=np.float32)
    target_ids = np.asarray(target_ids)
    emb_table = np.asarray(emb_table, dtype=np.float32)
    pos_table = np.asarray(pos_table, dtype=np.float32)

    body_nc, proj_nc = _get_kernels()

    mask = (np.arange(128)[:, None] <= np.arange(128)[None, :]).astype(BF)
    in_maps = []
    for b in range(B):
        x0 = emb_table[target_ids[b]] + pos_table[:S]
        in_maps.append(_body_in_map(
            x0, np.asarray(sa_wq, np.float32), np.asarray(sa_wk, np.float32),
            np.asarray(sa_wv, np.float32), np.asarray(sa_bq, np.float32),
            np.asarray(sa_bk, np.float32), np.asarray(sa_bv, np.float32),
            np.asarray(sa_wo, np.float32), np.asarray(sa_bo, np.float32),
            np.asarray(ca_wv, np.float32), np.asarray(ca_bv, np.float32),
            np.asarray(ca_wo, np.float32), np.asarray(ca_bo, np.float32),
            meaning[b],
            np.asarray(ln1_g, np.float32), np.asarray(ln1_b, np.float32),
            np.asarray(ln2_g, np.float32), np.asarray(ln2_b, np.float32),
            np.asarray(ln3_g, np.float32), np.asarray(ln3_b, np.float32),
            np.asarray(ff_w1, np.float32), np.asarray(ff_b1, np.float32),
            np.asarray(ff_w2, np.float32), np.asarray(ff_b2, np.float32), mask))
    body_res = run_bass_kernel_spmd(body_nc, in_maps, core_ids=list(range(B)))
    xT_all = np.concatenate([body_res.results[b]["xfT"] for b in range(B)], axis=1)  # [D, TOK] bf16

    w_out = np.asarray(w_out, dtype=np.float32)
    wT_pad = np.zeros((D, NCORES * VS), dtype=BF)
    wT_pad[:, :V] = w_out.T.astype(BF)
    proj_maps = [{"xT": xT_all, "wT": np.ascontiguousarray(wT_pad[:, c * VS:(c + 1) * VS])}
                 for c in range(NCORES)]
    proj_res = run_bass_kernel_spmd(proj_nc, proj_maps, core_ids=list(range(NCORES)))

    logits = np.empty((TOK, V), dtype=np.float32)
    for c in range(NCORES):
        lo = c * VS
        hi = min(lo + VS, V)
        if hi > lo:
            logits[:, lo:hi] = proj_res.results[c]["out"][:, :hi - lo].astype(np.float32)
    b_out = np.asarray(b_out, dtype=np.float32)
    if np.any(b_out):
        logits += b_out[None, :]
    return logits.reshape(B, S, V)
